# revision 23
# baseline (speedup 1.0000x reference)
"""Sigmoid-attention (DiffAttention) kernel for 8 Trainium2 NeuronCores.

Problem:  N=L=4096, H=8 heads, M=D=64.
    scores[n,l,h] = sigmoid(q[n,h,:] . k[l,h,:])
    out[n,h,:]    = (scores @ v) / sum_l(scores)        (per head)

Sharding: one head per core (8 heads == 8 cores). Each core gets its
head's Q/K transposed to [64, 4096] (duplicated onto both SBUF
partition halves; Q pre-scaled by 1/8) plus V packed as [V | ones]
tiles, computes the full attention for that head, and returns the head
output transposed ([64, 4096]); the host restores [4096, 8, 64].

Per-core dataflow (fp16 matmuls, fp32 PSUM accumulation):
    y^T[l,n]   = matmul(lhsT=K^T[:,l_tile], rhs=(Q/8)^T[:,n_chunk])
                 even l_tiles on PE rows 0-63, odd on 64-127 (2 streams)
    A^T[l,n]   = sigmoid(8*y^T): split between ACT (exact spline LUT,
                 scale=8) and DVE (two custom 8-stage ops approximating
                 sigmoid via (1-y+c*y^2)^8 = e^-x and a 1-Newton
                 reciprocal; max |err| 3e-3) so the two engines chew the
                 134M sigmoids concurrently -- ACT alone is the
                 baseline bottleneck.
    acc       += matmul(lhsT=[V|1][l_tile], rhs=A^T[l_tile])   (K=128)
    out        = acc[0:64] / acc[64]  (DVE recip + GpSimd bcast + DVE mul)
"""

from contextlib import ExitStack

import numpy as np

import concourse.bass as bass
import concourse.mybir as mybir
import concourse.tile as tile
from concourse import bacc
from concourse.bass import ts
from concourse.bass_utils import run_bass_kernel_spmd

N, L, H, M, D = 4096, 4096, 8, 64, 64
NCORES = 8
NCHUNK = 1024  # n columns per PSUM chunk
NCHUNKS = N // NCHUNK
LTILES = L // 128
VW = D + 1  # V columns + ones column
SKEW = 6  # mm2 trails sigmoid by SKEW l_tiles so PE always has ready work
CDT = mybir.dt.float16  # PE input dtype
FP32 = mybir.dt.float32
SIGMOID = mybir.ActivationFunctionType.Sigmoid
QSCALE = 8.0  # Q pre-scaled by 1/QSCALE on host; ACT un-scales, DVE wants y=s/8

# DVE sigmoid constants (fit numerically: rms err 1.1e-3, max 2.9e-3)
EXP8_CLAMP_LO = -2.0
EXP8_CURV = 0.5025
RECIP_C0 = -0.24
RECIP_C1 = 2.005

# Pair-level sigmoid engine split: the even (e) tile of each mm1 pair always
# goes to ACT; the odd (o) tile goes to DVE for these pair indices (mod 16).
# 9/16 pairs -> 28% of tiles on DVE; max run of 2 DVE-pairs so neither
# engine starves with only 3 in-flight score buffers.
DVE_PAIRS = frozenset({0, 1, 3, 4, 6, 8, 10, 12, 14})
GPSIMD_MUL = False  # gpsimd tensor_mul fails walrus codegen; keep mul on DVE

_CACHE: dict = {}
DEBUG = False


def _register_dve_ops():
    """Register the two custom DVE sigmoid ops (idempotent)."""
    import concourse.dve_ops as dmod
    from concourse.dve_ops import DveOp, has_src1
    from concourse.dve_spec import (
        Spec, Src0, C0, C1, One, Bin, AluOp, sq, maxx, minn, lower,
    )
    from concourse.dve_table_gen import dve_ver_for, DveOpSpec

    if "nn_exp8" in dmod._SUB_OPCODE_FOR_NAME:
        by_name = {op.name: op for op in dmod.OPS}
        return by_name["nn_exp8"], by_name["nn_sigrecip1"]

    # op1: out = ((1-ym) + C1*ym^2)^4 with ym = clamp(in0, C0, 1).
    # For in0 = x/8 this is e^(-x/2)-ish; squared again in op2 -> e^-x.
    ym = maxx(minn(Src0, One), C0)
    p = (One - ym) + sq(ym) * C1
    p2 = sq(p)
    body1 = sq(p2)

    def ref_exp8(in0, in1, s0, s1, imm2):
        ymr = np.maximum(np.minimum(in0.astype(np.float32), 1.0), s0)
        pr = (1.0 - ymr) + s1 * ymr * ymr
        return ((pr * pr) * (pr * pr)).astype(np.float32)

    # op2: u = 1 + sq(in0) (= 1+e^-x); seed = bitnot(u)*C0; one Newton step.
    u = One + sq(Src0)
    nt = Bin(AluOp.BITWISE_NOT, u, u)
    y0 = nt * C0
    body2 = y0 * (C1 - u * y0)

    def ref_sigr1(in0, in1, s0, s1, imm2):
        x = in0.astype(np.float32)
        uu = 1.0 + x * x
        y0r = (~uu.view(np.int32)).view(np.float32) * s0
        return (y0r * (s1 - uu * y0r)).astype(np.float32)

    ops = []
    for name, body, ref in (
        ("nn_exp8", body1, ref_exp8),
        ("nn_sigrecip1", body2, ref_sigr1),
    ):
        spec = Spec(body=body, reference=ref)
        row = dmod._CUSTOM_DVE_ROW_BASE + len(dmod.OPS)
        shas = {}
        for ver in ("v3", "v4"):
            try:
                tmp = DveOpSpec(
                    name=name, opcode=row, uops=lower(spec, ver=ver),
                    rd1_en=has_src1(spec),
                )
                shas[ver] = tmp.sha(ver)
            except Exception:
                pass
        op = DveOp(name, spec, subdim=False, uops_sha=shas)
        dmod.OPS.append(op)
        dmod.CUSTOM_DVE_SPECS[name] = spec
        dmod._SUB_OPCODE_FOR_NAME[name] = row
        ops.append(op)
    return ops[0], ops[1]


def build_nc():
    op_exp8, op_sigr1 = _register_dve_ops()

    nc = bacc.Bacc("TRN2", target_bir_lowering=False, debug=False)

    q2_d = nc.dram_tensor("q2", [128, N], CDT, kind="ExternalInput").ap()
    k2_d = nc.dram_tensor("k2", [128, L], CDT, kind="ExternalInput").ap()
    v1_d = nc.dram_tensor("v1", [128, LTILES * VW], CDT, kind="ExternalInput").ap()
    out_d = nc.dram_tensor("out", [D, N], FP32, kind="ExternalOutput").ap()
    if DEBUG:
        dbg_acc = nc.dram_tensor("dbg_acc", [VW, N], FP32, kind="ExternalOutput").ap()
        dbg_rec = nc.dram_tensor("dbg_rec", [1, N], FP32, kind="ExternalOutput").ap()
        dbg_bc = nc.dram_tensor("dbg_bc", [D, N], FP32, kind="ExternalOutput").ap()

    with ExitStack() as ctx:
        tc = ctx.enter_context(tile.TileContext(nc))
        const = ctx.enter_context(tc.tile_pool(name="const", bufs=1))
        apool = ctx.enter_context(tc.tile_pool(name="apool", bufs=SKEW + 2))
        epool = ctx.enter_context(tc.tile_pool(name="epool", bufs=2))
        io = ctx.enter_context(tc.tile_pool(name="io", bufs=2))
        psA = ctx.enter_context(tc.tile_pool(name="psA", bufs=3, space="PSUM"))
        psAcc = ctx.enter_context(tc.tile_pool(name="psAcc", bufs=1, space="PSUM"))

        # Split the input loads so the first l_tiles / n-chunks unblock early.
        q2_s = const.tile([128, N], CDT)
        k2_s = const.tile([128, L], CDT)
        v1_s = const.tile([128, LTILES * VW], CDT)
        for ci in range(NCHUNKS):
            cs = ci * NCHUNK
            nc.sync.dma_start(out=k2_s[:, cs : cs + NCHUNK], in_=k2_d[:, cs : cs + NCHUNK])
            nc.sync.dma_start(
                out=v1_s[:, ci * 8 * VW : (ci + 1) * 8 * VW],
                in_=v1_d[:, ci * 8 * VW : (ci + 1) * 8 * VW],
            )
            nc.sync.dma_start(out=q2_s[:, cs : cs + NCHUNK], in_=q2_d[:, cs : cs + NCHUNK])

        def mm1pair(ci, lt, sT_e, sT_o):
            # even l_tile on rows 0-63, odd on 64-127; interleave the two
            # streams so they run concurrently. Same-weight matmuls are
            # adjacent per row-half to give walrus a dedup/overlap chance.
            cs = ci * NCHUNK
            ke = k2_s[0:64, ts(lt, 128)]
            ko = k2_s[64:128, ts(lt + 1, 128)]
            for h in range(NCHUNK // 512):
                qsl = slice(cs + h * 512, cs + (h + 1) * 512)
                nc.tensor.matmul(
                    sT_e[:, ts(h, 512)], ke, q2_s[0:64, qsl], start=True, stop=True
                )
                nc.tensor.matmul(
                    sT_o[:, ts(h, 512)], ko, q2_s[64:128, qsl], start=True, stop=True
                )

        def sig(use_dve, sT, aT):
            """Sigmoid of one l_tile: ACT (exact) or DVE (approx) path."""
            if use_dve:
                tmp = epool.tile([128, NCHUNK], CDT, tag="e8", name="e8")
                nc.vector._custom_dve(
                    op_exp8, out=tmp, in0=sT, s0=EXP8_CLAMP_LO, s1=EXP8_CURV
                )
                nc.vector._custom_dve(
                    op_sigr1, out=aT, in0=tmp, s0=RECIP_C0, s1=RECIP_C1
                )
            else:
                nc.scalar.activation(aT, sT, SIGMOID, scale=QSCALE)

        def mm2(lt, aT, acc):
            va = v1_s[:, lt * VW : (lt + 1) * VW]
            first, last = lt == 0, lt == LTILES - 1
            for h in range(NCHUNK // 512):
                hs = ts(h, 512)
                nc.tensor.matmul(acc[:, hs], va, aT[:, hs], start=first, stop=last)

        for ci in range(NCHUNKS):
            cs = ci * NCHUNK
            acc = psAcc.tile([VW, NCHUNK], FP32, tag="acc")
            aTs = [None] * LTILES

            def mm1sig(ci, lt):
                sT_e = psA.tile([128, NCHUNK], FP32, tag="sT", name="sT")
                sT_o = psA.tile([128, NCHUNK], FP32, tag="sT", name="sT")
                mm1pair(ci, lt, sT_e, sT_o)
                aTs[lt] = apool.tile([128, NCHUNK], CDT, tag="aT", name="aT")
                aTs[lt + 1] = apool.tile([128, NCHUNK], CDT, tag="aT", name="aT")
                pi = ci * (LTILES // 2) + lt // 2
                sig(False, sT_e, aTs[lt])
                sig((pi % 16) in DVE_PAIRS, sT_o, aTs[lt + 1])

            for lt in range(0, SKEW, 2):
                mm1sig(ci, lt)
            for lt in range(SKEW, LTILES, 2):
                # mm2 first: its sigmoid is SKEW iterations old, so PE never
                # stalls here; any wait lands on mm1 (sT slot).
                mm2(lt - SKEW, aTs[lt - SKEW], acc)
                mm2(lt - SKEW + 1, aTs[lt - SKEW + 1], acc)
                aTs[lt - SKEW] = aTs[lt - SKEW + 1] = None
                mm1sig(ci, lt)
            for lt in range(LTILES - SKEW, LTILES):
                mm2(lt, aTs[lt], acc)
                aTs[lt] = None

            # Epilogue: out = acc[0:D] / acc[D]. ACT (which has slack) copies
            # acc out first so the PSUM accumulator frees immediately and the
            # next chunk's mm2 can start without a ~3.5us PE stall.
            summ = io.tile([VW, NCHUNK], FP32, tag="summ")
            nc.scalar.copy(summ, acc)
            # Custom-DVE ops mis-handle base_partition>0 reads, so move the
            # normalizer row to partition 0 with a stock copy first.
            norm_sb = io.tile([1, NCHUNK], FP32, tag="norm")
            nc.vector.tensor_copy(norm_sb, summ[D : D + 1, :])
            rec = io.tile([1, NCHUNK], FP32, tag="rec")
            nc.vector.reciprocal_approx_fast(out=rec, in_=norm_sb)
            bc = io.tile([D, NCHUNK], FP32, tag="bc")
            nc.gpsimd.partition_broadcast(bc, rec, channels=D)
            o = io.tile([D, NCHUNK], FP32, tag="o")
            nc.vector.tensor_mul(o, summ[0:D, :], bc)
            nc.sync.dma_start(out=out_d[:, cs : cs + NCHUNK], in_=o)
            if DEBUG:
                dacc = io.tile([VW, NCHUNK], FP32, tag="dacc")
                nc.vector.tensor_copy(dacc, acc)
                nc.sync.dma_start(out=dbg_acc[:, cs : cs + NCHUNK], in_=dacc)
                nc.sync.dma_start(out=dbg_rec[:, cs : cs + NCHUNK], in_=rec)
                nc.sync.dma_start(out=dbg_bc[:, cs : cs + NCHUNK], in_=bc)

    nc.compile()
    return nc


def get_nc():
    if "nc" not in _CACHE:
        _CACHE["nc"] = build_nc()
    return _CACHE["nc"]


def make_in_maps(queries, keys, values):
    np_cdt = mybir.dt.np(CDT)
    in_maps = []
    for h in range(NCORES):
        qT = np.ascontiguousarray((queries[:, h, :].T / QSCALE).astype(np_cdt))
        kT = np.ascontiguousarray(keys[:, h, :].T.astype(np_cdt))
        v1 = np.empty((L, VW), np_cdt)
        v1[:, :D] = values[:, h, :]
        v1[:, D] = 1.0
        v1p = np.ascontiguousarray(
            v1.reshape(LTILES, 128, VW).transpose(1, 0, 2).reshape(128, LTILES * VW)
        )
        in_maps.append(
            {
                "q2": np.vstack([qT, qT]),
                "k2": np.vstack([kT, kT]),
                "v1": v1p,
            }
        )
    return in_maps


def run(queries, keys, values, trace=False):
    """Returns (out [N,H,D] fp32, BassKernelResults)."""
    nc = get_nc()
    in_maps = make_in_maps(queries, keys, values)
    res = run_bass_kernel_spmd(nc, in_maps, core_ids=list(range(NCORES)), trace=trace)
    out = np.empty((N, H, D), np.float32)
    for h in range(NCORES):
        out[:, h, :] = res.results[h]["out"].T
    return out, res


def kernel(queries, keys, values):
    out, _ = run(np.asarray(queries), np.asarray(keys), np.asarray(values))
    return out
